# revision 11
# baseline (speedup 1.0000x reference)
"""Trainium2 Bass kernel for nn_CustomConv2D: gather 16x16 patches at given
centers and apply a shared [768 -> 1024] linear projection + bias.

Sharding: data-parallel over batch across 8 NeuronCores (8 images/core,
4608 patches/core); weight+bias replicated.

Host prepares im2col patches in k-major layout ([128, 6, NPC]: contraction
on partitions); the device then runs a pure accumulating-matmul pipeline:
per 128-patch block, 12 matmuls [128x128 (f32r) @ 128x512] accumulate
out[128 patches, 1024] over the 6 k-slices in PSUM, bias-add on DVE, DMA
out. The patch extraction runs on host: TRN2's SWDGE indirect-DMA costs
~1.4us/instruction with one descriptor per partition (measured), so any
device-side gather of 221k 64B patch rows is ~2.4ms -- off the roofline,
while the projection itself is compute/HBM co-bound at ~100us/core.

Matmul dtype: float32r (TRN2 fast-fp32, 1 cycle/row vs 4 for fp32,
~1.5e-4 relative rounding). Set CONV_MM_DT=f32 for exact fp32 (4x PE).
"""

import os
import numpy as np

import concourse.bass as bass
from concourse import bacc
import concourse.mybir as mybir
import concourse.tile as tile

# problem shape (hardcoded per contract)
B, C, H, W = 64, 3, 384, 384
N, K, O = 576, 16, 1024
NCORES = 8
B_LOC = B // NCORES          # 8 images per core
NPC = B_LOC * N              # 4608 patches per core
P = 128                      # partitions / patches per block
NBLK = NPC // P              # 36 blocks
KDIM = C * K * K             # 768 contraction dim
KSL = KDIM // P              # 6 k-slices

MM_DT = os.environ.get("CONV_MM_DT", "f32r")


def _build(reps: int = 1):
    nc = bacc.Bacc()
    f32 = mybir.dt.float32
    mm_dt = f32 if MM_DT == "f32" else mybir.dt.float32r

    gt_t = nc.declare_dram_parameter("gt", [P, KSL, NPC], mm_dt, isOutput=False)
    wt_t = nc.declare_dram_parameter("wt", [P, KSL, O], mm_dt, isOutput=False)
    bias_t = nc.declare_dram_parameter("bias", [1, O], f32, isOutput=False)
    out_t = nc.declare_dram_parameter("out", [NPC, O], f32, isOutput=True)

    with tile.TileContext(nc) as tc:
        with (
            tc.tile_pool(name="const", bufs=1) as cpool,
            tc.tile_pool(name="gt", bufs=4) as gtpool,
            tc.tile_pool(name="osb", bufs=4) as opool,
            tc.tile_pool(name="outp", bufs=4, space="PSUM") as outppool,
        ):
            # weights chunked per k-slice; slice 0 is issued before the first
            # patch load so the first matmuls start ~2us in, the rest follow
            # interleaved behind block 0's patches on the same ring
            wt_sb = cpool.tile([P, KSL, O], mm_dt)
            nc.sync.dma_start(wt_sb[:, 0, :512], wt_t[:, 0, :512])
            nc.sync.dma_start(wt_sb[:, 0, 512:], wt_t[:, 0, 512:])
            # bias broadcast [1,O] -> [128,O] on the (otherwise idle) GpSimd
            bias_row = cpool.tile([1, O], f32)
            nc.gpsimd.dma_start(bias_row[:], bias_t[:])
            bias_bc = cpool.tile([P, O], f32)
            nc.gpsimd.partition_broadcast(bias_bc[:], bias_row[:])

            # HAM warm-up: ~4us of junk matmuls while the first loads land,
            # so the PE clock is unthrottled (2.4GHz) when real work starts
            ones_w = cpool.tile([1, P], f32)
            nc.vector.memset(ones_w[:], 1.0)
            warm_ps = outppool.tile([P, O], f32, tag="outp")
            for _ in range(12):
                nc.tensor.matmul(warm_ps[:1, :P], lhsT=ones_w[:, :1],
                                 rhs=ones_w[:], start=True, stop=True)

            def body(_i=None):
                for t in range(NBLK):
                    # k-major patch tile for this block: [128k, 6, 128p]
                    gt_sb = gtpool.tile([P, KSL, P], mm_dt, tag="gt")
                    nc.sync.dma_start(gt_sb[:], gt_t[:, :, t * P:(t + 1) * P])
                    if t == 0:
                        for ks in range(1, KSL):
                            nc.sync.dma_start(wt_sb[:, ks, :], wt_t[:, ks, :])
                    out_ps = outppool.tile([P, O], f32, tag="outp")
                    # k-slice outer: both halves consume a weight chunk right
                    # after it lands, so block 0 isn't gated on the full tensor
                    for ks in range(KSL):
                        for h in range(O // 512):
                            hs = slice(h * 512, (h + 1) * 512)
                            nc.tensor.matmul(
                                out_ps[:, hs],
                                lhsT=gt_sb[:, ks, :],
                                rhs=wt_sb[:, ks, hs],
                                start=(ks == 0), stop=(ks == KSL - 1),
                            )
                    # bias-add + store per 512-half: the first half's store
                    # (on the second HWDGE ring) overlaps the second half's
                    # add, and stores stay off the patch-load ring
                    o_sb = opool.tile([P, O], f32, tag="osb")
                    for h in range(O // 512):
                        hs = slice(h * 512, (h + 1) * 512)
                        nc.vector.tensor_add(o_sb[:, hs], out_ps[:, hs],
                                             bias_bc[:, hs])
                        nc.scalar.dma_start(out_t[t * P:(t + 1) * P, hs],
                                            o_sb[:, hs])

            if reps == 1:
                body()
            else:
                with tc.For_i(0, reps, 1) as i:
                    body(i)
    nc.finalize()
    return nc


_CACHE = {}


def _get_nc(reps: int = 1):
    if reps not in _CACHE:
        _CACHE[reps] = _build(reps)
    return _CACHE[reps]


def _prep_inputs(x, centers, weight, bias):
    x = np.ascontiguousarray(x, dtype=np.float32)
    centers = np.asarray(centers, dtype=np.int64)
    weight = np.ascontiguousarray(weight, dtype=np.float32)
    bias = np.ascontiguousarray(bias, dtype=np.float32)

    # host im2col: patches [B, N, C*K*K]
    win = np.lib.stride_tricks.sliding_window_view(x, (K, K), axis=(2, 3))
    r0 = centers[:, :, 0] - K // 2        # [B, N]
    c0 = centers[:, :, 1] - K // 2
    b_ids = np.arange(B)[:, None]
    patches = win[b_ids, :, r0, c0]       # [B, N, C, K, K]

    # weight [O, C, K, K] -> wT [KDIM, O] -> [128, KSL, O]
    wflat = weight.reshape(O, KDIM)
    wt_host = np.ascontiguousarray(
        wflat.T.reshape(KSL, P, O).transpose(1, 0, 2))
    bias_host = bias.reshape(1, O)

    in_maps = []
    for core in range(NCORES):
        pc = patches[core * B_LOC:(core + 1) * B_LOC].reshape(NPC, KDIM)
        # k-major: gt[p, ks, n] = patch n element ks*128+p
        gt_host = np.ascontiguousarray(
            pc.T.reshape(KSL, P, NPC).transpose(1, 0, 2))
        in_maps.append({"gt": gt_host, "wt": wt_host, "bias": bias_host})
    return in_maps


def kernel(x, centers, weight, bias):
    from concourse.bass_utils import run_bass_kernel_spmd
    nc = _get_nc(1)
    in_maps = _prep_inputs(x, centers, weight, bias)
    res = run_bass_kernel_spmd(nc, in_maps, list(range(NCORES))).results
    out = np.stack([res[i]["out"] for i in range(NCORES)], axis=0)
    return out.reshape(B, N, O)


# revision 12
# speedup vs baseline: 1.1266x; 1.1266x over previous
"""Trainium2 Bass kernel for nn_CustomConv2D: gather 16x16 patches at given
centers and apply a shared [768 -> 1024] linear projection + bias.

Sharding: data-parallel over batch across 8 NeuronCores (8 images/core,
4608 patches/core); weight+bias replicated.

Host prepares im2col patches in k-major layout ([128, 6, NPC]: contraction
on partitions); the device then runs a pure accumulating-matmul pipeline:
per 128-patch block, 12 matmuls [128x128 (f32r) @ 128x512] accumulate
out[128 patches, 1024] over the 6 k-slices in PSUM, bias-add on DVE, DMA
out. The patch extraction runs on host: TRN2's SWDGE indirect-DMA costs
~1.4us/instruction with one descriptor per partition (measured), so any
device-side gather of 221k 64B patch rows is ~2.4ms -- off the roofline,
while the projection itself is compute/HBM co-bound at ~100us/core.

Matmul dtype: float32r (TRN2 fast-fp32, 1 cycle/row vs 4 for fp32,
~1.5e-4 relative rounding). Set CONV_MM_DT=f32 for exact fp32 (4x PE).
"""

import os
import numpy as np

import concourse.bass as bass
from concourse import bacc
import concourse.mybir as mybir
import concourse.tile as tile

# problem shape (hardcoded per contract)
B, C, H, W = 64, 3, 384, 384
N, K, O = 576, 16, 1024
NCORES = 8
B_LOC = B // NCORES          # 8 images per core
NPC = B_LOC * N              # 4608 patches per core
P = 128                      # partitions / patches per block
NBLK = NPC // P              # 36 blocks
KDIM = C * K * K             # 768 contraction dim
KSL = KDIM // P              # 6 k-slices

MM_DT = os.environ.get("CONV_MM_DT", "f32r")


def _build(reps: int = 1):
    nc = bacc.Bacc()
    f32 = mybir.dt.float32
    mm_dt = f32 if MM_DT == "f32" else mybir.dt.float32r

    gt_t = nc.declare_dram_parameter("gt", [P, KSL, NPC], mm_dt, isOutput=False)
    wt_t = nc.declare_dram_parameter("wt", [P, KSL, O], mm_dt, isOutput=False)
    bias_t = nc.declare_dram_parameter("bias", [1, O], f32, isOutput=False)
    out_t = nc.declare_dram_parameter("out", [NPC, O], f32, isOutput=True)

    with tile.TileContext(nc) as tc:
        with (
            tc.tile_pool(name="const", bufs=1) as cpool,
            tc.tile_pool(name="gt", bufs=4) as gtpool,
            tc.tile_pool(name="osb", bufs=4) as opool,
            tc.tile_pool(name="outp", bufs=4, space="PSUM") as outppool,
        ):
            # weights chunked per k-slice; slice 0 is issued before the first
            # patch load so the first matmuls start ~2us in, the rest follow
            # interleaved behind block 0's patches on the same ring
            wt_sb = cpool.tile([P, KSL, O], mm_dt)
            nc.sync.dma_start(wt_sb[:, 0, :512], wt_t[:, 0, :512])
            nc.sync.dma_start(wt_sb[:, 0, 512:], wt_t[:, 0, 512:])
            # bias broadcast [1,O] -> [128,O] on the (otherwise idle) GpSimd
            bias_row = cpool.tile([1, O], f32)
            nc.gpsimd.dma_start(bias_row[:], bias_t[:])
            bias_bc = cpool.tile([P, O], f32)
            nc.gpsimd.partition_broadcast(bias_bc[:], bias_row[:])

            def body(_i=None):
                for t in range(NBLK):
                    # k-major patch tile for this block: [128k, 6, 128p]
                    gt_sb = gtpool.tile([P, KSL, P], mm_dt, tag="gt")
                    nc.sync.dma_start(gt_sb[:], gt_t[:, :, t * P:(t + 1) * P])
                    if t == 0:
                        for ks in range(1, KSL):
                            nc.sync.dma_start(wt_sb[:, ks, :], wt_t[:, ks, :])
                    out_ps = outppool.tile([P, O], f32, tag="outp")
                    # k-slice outer: both halves consume a weight chunk right
                    # after it lands, so block 0 isn't gated on the full tensor
                    for ks in range(KSL):
                        for h in range(O // 512):
                            hs = slice(h * 512, (h + 1) * 512)
                            nc.tensor.matmul(
                                out_ps[:, hs],
                                lhsT=gt_sb[:, ks, :],
                                rhs=wt_sb[:, ks, hs],
                                start=(ks == 0), stop=(ks == KSL - 1),
                            )
                    # bias-add + store per 512-half: the first half's store
                    # (on the second HWDGE ring) overlaps the second half's
                    # add, and stores stay off the patch-load ring
                    o_sb = opool.tile([P, O], f32, tag="osb")
                    for h in range(O // 512):
                        hs = slice(h * 512, (h + 1) * 512)
                        nc.vector.tensor_add(o_sb[:, hs], out_ps[:, hs],
                                             bias_bc[:, hs])
                        nc.scalar.dma_start(out_t[t * P:(t + 1) * P, hs],
                                            o_sb[:, hs])

            if reps == 1:
                body()
            else:
                with tc.For_i(0, reps, 1) as i:
                    body(i)
    nc.finalize()
    return nc


_CACHE = {}


def _get_nc(reps: int = 1):
    if reps not in _CACHE:
        _CACHE[reps] = _build(reps)
    return _CACHE[reps]


def _prep_inputs(x, centers, weight, bias):
    x = np.ascontiguousarray(x, dtype=np.float32)
    centers = np.asarray(centers, dtype=np.int64)
    weight = np.ascontiguousarray(weight, dtype=np.float32)
    bias = np.ascontiguousarray(bias, dtype=np.float32)

    # host im2col: patches [B, N, C*K*K]
    win = np.lib.stride_tricks.sliding_window_view(x, (K, K), axis=(2, 3))
    r0 = centers[:, :, 0] - K // 2        # [B, N]
    c0 = centers[:, :, 1] - K // 2
    b_ids = np.arange(B)[:, None]
    patches = win[b_ids, :, r0, c0]       # [B, N, C, K, K]

    # weight [O, C, K, K] -> wT [KDIM, O] -> [128, KSL, O]
    wflat = weight.reshape(O, KDIM)
    wt_host = np.ascontiguousarray(
        wflat.T.reshape(KSL, P, O).transpose(1, 0, 2))
    bias_host = bias.reshape(1, O)

    in_maps = []
    for core in range(NCORES):
        pc = patches[core * B_LOC:(core + 1) * B_LOC].reshape(NPC, KDIM)
        # k-major: gt[p, ks, n] = patch n element ks*128+p
        gt_host = np.ascontiguousarray(
            pc.T.reshape(KSL, P, NPC).transpose(1, 0, 2))
        in_maps.append({"gt": gt_host, "wt": wt_host, "bias": bias_host})
    return in_maps


def kernel(x, centers, weight, bias):
    from concourse.bass_utils import run_bass_kernel_spmd
    nc = _get_nc(1)
    in_maps = _prep_inputs(x, centers, weight, bias)
    res = run_bass_kernel_spmd(nc, in_maps, list(range(NCORES))).results
    out = np.stack([res[i]["out"] for i in range(NCORES)], axis=0)
    return out.reshape(B, N, O)
